# revision 9
# baseline (speedup 1.0000x reference)
"""HGATConv (4-head graph attention, N=4096, F=512) on 8 Trainium2 NeuronCores.

Sharding: (head, node-half) grid — core c handles head c//2 and output rows
q*2048..(q+1)*2048 (q = c%2). Every core computes its head's h = x @ W_h for
ALL nodes locally (bf16 matmuls, ~11us) — no collective at all.

Elementwise attention math per 128-node j-block (tiles are [128 j, 2048 i]):
  exp(leakyrelu(z)) = max(exp(z), exp(z/5)),  z = si[i] + sj[j]
scaled by exp(-sj/5) so the second branch is the jb-invariant tensor E2b:
  p' = max(exp(si + 0.8 sj), exp(si/5)) = p / exp(sj/5)
The exp(sj/5) factor is restored by scaling haug (129-wide) before the
weighted-sum matmul, which also scales the row-sum column consistently.
Per j-block DVE work is just 2 wide tensor_tensor ops (max, mask-mult) at
the 2x bf16 rate plus one tiny tensor_scalar — no 1x scalar_tensor_tensor.
"""

import sys
import numpy as np

if "/opt/trn_rl_repo" not in sys.path:
    sys.path.insert(0, "/opt/trn_rl_repo")

H, D = 4, 128          # heads, head dim
N, F = 4096, 512       # nodes, features
M = 8                  # cores
NOWN = 1024 * 2        # 2048 own output rows per core
JB = N // 128          # 32 j blocks
IB = NOWN // 128       # 16 own-row blocks
KB = F // 128          # 4 contraction blocks
DA = D + 2             # head W columns + wa1 + wa2

_CACHE = {}


def _build_nc():
    import concourse.bacc as bacc
    from concourse import mybir
    from concourse.tile import TileContext

    f32 = mybir.dt.float32
    bf16 = mybir.dt.bfloat16
    Alu = mybir.AluOpType
    Act = mybir.ActivationFunctionType

    nc = bacc.Bacc()
    xT_d = nc.declare_dram_parameter("xT", [F, N], bf16, isOutput=False)
    xq_d = nc.declare_dram_parameter("xq", [F, NOWN], bf16, isOutput=False)
    Wh_d = nc.declare_dram_parameter("Wh", [F, DA], bf16, isOutput=False)
    maskT_d = nc.declare_dram_parameter("maskT", [N, NOWN], bf16, isOutput=False)
    ones1_d = nc.declare_dram_parameter("ones1", [1, 128], f32, isOutput=False)
    out_d = nc.declare_dram_parameter("out", [NOWN, D], f32, isOutput=True)

    with TileContext(nc) as tc:
        with tc.tile_pool(name="const", bufs=1) as const_pool:
            xk = [const_pool.tile([128, N], bf16, name=f"xk{k}") for k in range(KB)]
            for k in range(KB):
                nc.sync.dma_start(xk[k][:], xT_d[k * 128:(k + 1) * 128, :])
            xq = [const_pool.tile([128, NOWN], bf16, name=f"xq{k}") for k in range(KB)]
            for k in range(KB):
                nc.sync.dma_start(xq[k][:], xq_d[k * 128:(k + 1) * 128, :])
            Wh_sb = const_pool.tile([128, KB * DA], bf16)
            for k in range(KB):
                nc.sync.dma_start(Wh_sb[:, k * DA:(k + 1) * DA],
                                  Wh_d[k * 128:(k + 1) * 128, :])
            ones1 = const_pool.tile([1, 128], f32)
            nc.sync.dma_start(ones1[:], ones1_d[:])

            haug = const_pool.tile([128, JB * (D + 1)], bf16)   # [h | 1] per block
            sjall = const_pool.tile([128, 2 * JB], f32)         # [s1 s2] per block
            f2all = const_pool.tile([128, JB], f32)             # exp(s2/5) per block
            sT_own = const_pool.tile([2, NOWN], f32)            # s1,s2 rows, own cols
            si_b = const_pool.tile([128, NOWN], f32)            # s1 bcast over parts
            E2b = const_pool.tile([128, NOWN], bf16)            # exp(s1/5) bcast

            # ---- stage A: h (all nodes) + per-node scores ----
            with tc.tile_pool(name="apsum", bufs=3, space="PSUM") as apsum:
                for b in range(JB):
                    ph = apsum.tile([128, DA], f32, tag="ph")
                    for k in range(KB):
                        nc.tensor.matmul(
                            ph[:],
                            lhsT=xk[k][:, b * 128:(b + 1) * 128],
                            rhs=Wh_sb[:, k * DA:(k + 1) * DA],
                            start=(k == 0), stop=(k == KB - 1))
                    nc.scalar.activation(haug[:, b * (D + 1):b * (D + 1) + D],
                                         ph[:, 0:D], Act.Copy)
                    nc.vector.tensor_copy(sjall[:, 2 * b:2 * b + 2], ph[:, D:D + 2])
                # ones columns for the row-sum ride-along
                haug3 = haug.rearrange("p (b c) -> p b c", c=D + 1)
                nc.vector.memset(haug3[:, :, D:D + 1], 1.0)

                # sT_own rows: [s1; s2] for own 2048 columns
                for c4 in range(NOWN // 512):
                    pst = apsum.tile([2, 512], f32, tag="pst")
                    for k in range(KB):
                        nc.tensor.matmul(
                            pst[:],
                            lhsT=Wh_sb[:, k * DA + D:k * DA + DA],
                            rhs=xq[k][:, c4 * 512:(c4 + 1) * 512],
                            start=(k == 0), stop=(k == KB - 1))
                    nc.vector.tensor_copy(sT_own[:, c4 * 512:(c4 + 1) * 512], pst[:])

            # f2all = exp(s2/5), one strided ACT op; sj08 = 0.8*s2 bias columns
            sj3 = sjall.rearrange("p (b s) -> p b s", s=2)
            f23 = f2all.rearrange("p (b s) -> p b s", s=1)
            nc.scalar.activation(f23[:, :, 0:1], sj3[:, :, 1:2], Act.Exp, scale=0.2)
            sj08 = const_pool.tile([128, JB], f32)
            sj083 = sj08.rearrange("p (b s) -> p b s", s=1)
            nc.vector.tensor_scalar(sj083[:, :, 0:1], in0=sj3[:, :, 1:2],
                                    scalar1=0.8, scalar2=None, op0=Alu.mult)

            # ---- stage B: broadcast si over partitions ----
            with tc.tile_pool(name="bpsum", bufs=1, space="PSUM") as bpsum:
                for t in range(4):
                    pb = bpsum.tile([128, 512], f32, tag=f"pb{t}")
                    for s in range(4):
                        ib = 4 * t + s
                        nc.tensor.matmul(
                            pb[:, s * 128:(s + 1) * 128],
                            lhsT=ones1[:],
                            rhs=sT_own[0:1, ib * 128:(ib + 1) * 128],
                            start=(s == 0), stop=(s == 3),
                            skip_group_check=True)
                    nc.scalar.activation(si_b[:, t * 512:(t + 1) * 512],
                                         pb[:], Act.Copy)
                    nc.scalar.activation(E2b[:, t * 512:(t + 1) * 512],
                                         pb[:], Act.Exp, scale=0.2)

            # ---- main attention loop ----
            with (
                tc.tile_pool(name="acc", bufs=1, space="PSUM") as acc_pool,
                tc.tile_pool(name="stream", bufs=3) as stream,
                tc.tile_pool(name="tail", bufs=1) as tail_pool,
            ):
                acc = [acc_pool.tile([128, 2 * (D + 1)], f32, name=f"acc{t}")
                       for t in range(8)]

                for jb in range(JB):
                    mask = stream.tile([128, NOWN], bf16, tag="mask")
                    nc.sync.dma_start(mask[:], maskT_d[jb * 128:(jb + 1) * 128, :])

                    # t1' = exp(si + 0.8 sj)  (exp(z)/f2 with f2 = exp(sj/5))
                    t1 = stream.tile([128, NOWN], bf16, tag="t1")
                    nc.scalar.activation(t1[:], si_b[:], Act.Exp,
                                         bias=sj08[:, jb:jb + 1], scale=1.0)
                    p = stream.tile([128, NOWN], bf16, tag="p")
                    nc.vector.tensor_tensor(p[:], E2b[:], t1[:], op=Alu.max)
                    pm = stream.tile([128, NOWN], bf16, tag="pm")
                    nc.vector.tensor_tensor(pm[:], p[:], mask[:], op=Alu.mult)

                    # haug scaled by f2 (restores the exp(sj/5) factor)
                    hsc = stream.tile([128, D + 1], bf16, tag="hsc")
                    nc.vector.tensor_scalar(
                        hsc[:], in0=haug[:, jb * (D + 1):(jb + 1) * (D + 1)],
                        scalar1=f2all[:, jb:jb + 1], scalar2=None, op0=Alu.mult)

                    for ib in range(IB):
                        t8, s = divmod(ib, 2)
                        nc.tensor.matmul(
                            acc[t8][:, s * (D + 1):(s + 1) * (D + 1)],
                            lhsT=pm[:, ib * 128:(ib + 1) * 128],
                            rhs=hsc[:],
                            start=(jb == 0 and s == 0),
                            stop=(jb == JB - 1 and s == 1),
                            skip_group_check=True)

                # ---- tail: normalize + elu + store ----
                osb = tail_pool.tile([128, NOWN], f32, tag="osb")
                rinv = tail_pool.tile([128, IB], f32, tag="rinv")
                for ib in range(IB):
                    t8, s = divmod(ib, 2)
                    nc.vector.reciprocal(
                        rinv[:, ib:ib + 1],
                        acc[t8][:, s * (D + 1) + D:s * (D + 1) + D + 1])
                for ib in range(IB):
                    t8, s = divmod(ib, 2)
                    nc.vector.tensor_scalar(
                        osb[:, ib * 128:(ib + 1) * 128],
                        in0=acc[t8][:, s * (D + 1):s * (D + 1) + D],
                        scalar1=rinv[:, ib:ib + 1], scalar2=None, op0=Alu.mult)
                # elu(x) = (relu(x) - 1) + exp(min(x, 0))
                zmin = tail_pool.tile([128, NOWN], f32, tag="zmin")
                nc.vector.tensor_scalar(zmin[:], in0=osb[:], scalar1=0.0,
                                        scalar2=None, op0=Alu.min)
                ez = tail_pool.tile([128, NOWN], f32, tag="ez")
                nc.scalar.activation(ez[:], zmin[:], Act.Exp)
                rm1 = tail_pool.tile([128, NOWN], f32, tag="rm1")
                nc.vector.tensor_scalar(rm1[:], in0=osb[:], scalar1=0.0,
                                        scalar2=-1.0, op0=Alu.max, op1=Alu.add)
                oo = tail_pool.tile([128, NOWN], f32, tag="oo")
                nc.vector.tensor_tensor(oo[:], ez[:], rm1[:], op=Alu.add)
                for ib in range(IB):
                    nc.sync.dma_start(out_d[ib * 128:(ib + 1) * 128, :],
                                      oo[:, ib * 128:(ib + 1) * 128])

    nc.compile()
    return nc


def _host_prep(x, adj, W, a):
    import ml_dtypes
    x = np.asarray(x, np.float32)
    adj = np.asarray(adj)
    W = np.asarray(W, np.float32)
    a = np.asarray(a, np.float32)

    xT = np.ascontiguousarray(x.T.astype(ml_dtypes.bfloat16))          # [F, N]
    adjT = np.ascontiguousarray(adj.T.astype(ml_dtypes.bfloat16))      # [j, i]
    ones1 = np.ones((1, 128), np.float32)

    in_maps = []
    for c in range(M):
        hd, q = divmod(c, 2)
        Wh = W[:, hd * D:(hd + 1) * D]                                  # [F, D]
        wa1 = Wh @ a[:D, 0]
        wa2 = Wh @ a[D:, 0]
        Whc = np.concatenate([Wh, wa1[:, None], wa2[:, None]], axis=1)  # [F, D+2]
        in_maps.append({
            "xT": xT,
            "xq": np.ascontiguousarray(xT[:, q * NOWN:(q + 1) * NOWN]),
            "Wh": np.ascontiguousarray(Whc.astype(ml_dtypes.bfloat16)),
            "maskT": np.ascontiguousarray(adjT[:, q * NOWN:(q + 1) * NOWN]),
            "ones1": ones1,
        })
    return in_maps


def kernel(x, adj, W, a):
    from concourse.bass_utils import run_bass_kernel_spmd

    if "nc" not in _CACHE:
        _CACHE["nc"] = _build_nc()
    nc = _CACHE["nc"]

    in_maps = _host_prep(x, adj, W, a)
    res = run_bass_kernel_spmd(nc, in_maps, list(range(M)))
    out = np.empty((N, H * D), np.float32)
    for c in range(M):
        hd, q = divmod(c, 2)
        out[q * NOWN:(q + 1) * NOWN, hd * D:(hd + 1) * D] = np.asarray(
            res.results[c]["out"], np.float32)
    return out


if __name__ == "__main__":
    nc = _build_nc()
    print("built ok")


# revision 14
# speedup vs baseline: 1.1748x; 1.1748x over previous
"""HGATConv (4-head graph attention, N=4096, F=512) on 8 Trainium2 NeuronCores.

Sharding: (head, node-half) grid — core c handles head c//2 and output rows
q*2048..(q+1)*2048 (q = c%2). Every core computes its head's h = x @ W_h for
ALL nodes locally (bf16 matmuls, ~11us) — no collective at all.

Elementwise attention math per 128-node j-block (tiles are [128 j, 2048 i]):
  exp(leakyrelu(z)) = max(exp(z), exp(z/5)),  z = si[i] + sj[j]
scaled by exp(-sj/5) so the second branch is the jb-invariant tensor E2b:
  p' = max(exp(si + 0.8 sj), exp(si/5)) = p / exp(sj/5)
The exp(sj/5) factor is restored by scaling haug (129-wide) before the
weighted-sum matmul, which also scales the row-sum column consistently.
Per j-block DVE work is just 2 wide tensor_tensor ops (max, mask-mult) at
the 2x bf16 rate plus one tiny tensor_scalar — no 1x scalar_tensor_tensor.
"""

import sys
import numpy as np

if "/opt/trn_rl_repo" not in sys.path:
    sys.path.insert(0, "/opt/trn_rl_repo")

H, D = 4, 128          # heads, head dim
N, F = 4096, 512       # nodes, features
M = 8                  # cores
NOWN = 1024 * 2        # 2048 own output rows per core
JB = N // 128          # 32 j blocks
IB = NOWN // 128       # 16 own-row blocks
KB = F // 128          # 4 contraction blocks
DA = D + 2             # head W columns + wa1 + wa2

_CACHE = {}


def _build_nc():
    import concourse.bacc as bacc
    from concourse import mybir
    from concourse.tile import TileContext

    f32 = mybir.dt.float32
    bf16 = mybir.dt.bfloat16
    Alu = mybir.AluOpType
    Act = mybir.ActivationFunctionType

    nc = bacc.Bacc()
    xT_d = nc.declare_dram_parameter("xT", [F, N], bf16, isOutput=False)
    xq_d = nc.declare_dram_parameter("xq", [F, NOWN], bf16, isOutput=False)
    Wh_d = nc.declare_dram_parameter("Wh", [F, DA], bf16, isOutput=False)
    maskT_d = nc.declare_dram_parameter("maskT", [N, NOWN], bf16, isOutput=False)
    ones1_d = nc.declare_dram_parameter("ones1", [1, 128], f32, isOutput=False)
    out_d = nc.declare_dram_parameter("out", [NOWN, D], f32, isOutput=True)

    with TileContext(nc) as tc:
        with tc.tile_pool(name="const", bufs=1) as const_pool:
            xk = [const_pool.tile([128, N], bf16, name=f"xk{k}") for k in range(KB)]
            for k in range(KB):
                nc.sync.dma_start(xk[k][:], xT_d[k * 128:(k + 1) * 128, :])
            xq = [const_pool.tile([128, NOWN], bf16, name=f"xq{k}") for k in range(KB)]
            for k in range(KB):
                nc.sync.dma_start(xq[k][:], xq_d[k * 128:(k + 1) * 128, :])
            Wh_sb = const_pool.tile([128, KB * DA], bf16)
            for k in range(KB):
                nc.sync.dma_start(Wh_sb[:, k * DA:(k + 1) * DA],
                                  Wh_d[k * 128:(k + 1) * 128, :])
            ones1 = const_pool.tile([1, 128], f32)
            nc.sync.dma_start(ones1[:], ones1_d[:])

            haug = const_pool.tile([128, JB * (D + 1)], bf16)   # [h | 1] per block
            sjall = const_pool.tile([128, 2 * JB], f32)         # [s1 s2] per block
            e1all = const_pool.tile([128, JB], f32)             # exp(s2) per block
            sT_own = const_pool.tile([2, NOWN], f32)            # s1,s2 rows, own cols
            si_b = const_pool.tile([128, NOWN], f32)            # s1 bcast over parts

            # ---- stage A: h (all nodes) + per-node scores ----
            with tc.tile_pool(name="apsum", bufs=3, space="PSUM") as apsum:
                for b in range(JB):
                    ph = apsum.tile([128, DA], f32, tag="ph")
                    for k in range(KB):
                        nc.tensor.matmul(
                            ph[:],
                            lhsT=xk[k][:, b * 128:(b + 1) * 128],
                            rhs=Wh_sb[:, k * DA:(k + 1) * DA],
                            start=(k == 0), stop=(k == KB - 1))
                    nc.scalar.activation(haug[:, b * (D + 1):b * (D + 1) + D],
                                         ph[:, 0:D], Act.Copy)
                    nc.vector.tensor_copy(sjall[:, 2 * b:2 * b + 2], ph[:, D:D + 2])
                # ones columns for the row-sum ride-along
                haug3 = haug.rearrange("p (b c) -> p b c", c=D + 1)
                nc.vector.memset(haug3[:, :, D:D + 1], 1.0)

                # sT_own rows: [s1; s2] for own 2048 columns
                for c4 in range(NOWN // 512):
                    pst = apsum.tile([2, 512], f32, tag="pst")
                    for k in range(KB):
                        nc.tensor.matmul(
                            pst[:],
                            lhsT=Wh_sb[:, k * DA + D:k * DA + DA],
                            rhs=xq[k][:, c4 * 512:(c4 + 1) * 512],
                            start=(k == 0), stop=(k == KB - 1))
                    nc.vector.tensor_copy(sT_own[:, c4 * 512:(c4 + 1) * 512], pst[:])

            # e1all = exp(s2), one strided ACT op; sj02 = 0.2*s2 bias columns.
            # Scores are divided by exp(s1_i) (row-constant, cancels in the
            # softmax): p'' = max(exp(-0.8 s1 + 0.2 s2), exp(s2)).
            sj3 = sjall.rearrange("p (b s) -> p b s", s=2)
            e13 = e1all.rearrange("p (b s) -> p b s", s=1)
            nc.scalar.activation(e13[:, :, 0:1], sj3[:, :, 1:2], Act.Exp)
            sj02 = const_pool.tile([128, JB], f32)
            sj023 = sj02.rearrange("p (b s) -> p b s", s=1)
            nc.vector.tensor_scalar(sj023[:, :, 0:1], in0=sj3[:, :, 1:2],
                                    scalar1=0.2, scalar2=None, op0=Alu.mult)

            # ---- stage B: broadcast si over partitions ----
            with tc.tile_pool(name="bpsum", bufs=1, space="PSUM") as bpsum:
                for t in range(4):
                    pb = bpsum.tile([128, 512], f32, tag=f"pb{t}")
                    for s in range(4):
                        ib = 4 * t + s
                        nc.tensor.matmul(
                            pb[:, s * 128:(s + 1) * 128],
                            lhsT=ones1[:],
                            rhs=sT_own[0:1, ib * 128:(ib + 1) * 128],
                            start=(s == 0), stop=(s == 3),
                            skip_group_check=True)
                    nc.scalar.activation(si_b[:, t * 512:(t + 1) * 512],
                                         pb[:], Act.Copy)

            # ---- main attention loop ----
            with (
                tc.tile_pool(name="acc", bufs=1, space="PSUM") as acc_pool,
                tc.tile_pool(name="stream", bufs=3) as stream,
                tc.tile_pool(name="tail", bufs=1) as tail_pool,
            ):
                acc = [acc_pool.tile([128, 2 * (D + 1)], f32, name=f"acc{t}")
                       for t in range(8)]

                for jb in range(JB):
                    mask = stream.tile([128, NOWN], bf16, tag="mask")
                    nc.sync.dma_start(mask[:], maskT_d[jb * 128:(jb + 1) * 128, :])

                    # t2 = exp(-0.8 s1_i + 0.2 s2_j) = exp(z/5) / exp(s1_i)
                    t2 = stream.tile([128, NOWN], bf16, tag="t2")
                    nc.scalar.activation(t2[:], si_b[:], Act.Exp,
                                         bias=sj02[:, jb:jb + 1], scale=-0.8)
                    # q1 = max(t2, exp(s2_j))  (= exp(leakyrelu(z))/exp(s1_i))
                    q1 = stream.tile([128, NOWN], bf16, tag="q1")
                    nc.vector.tensor_scalar(q1[:], in0=t2[:],
                                            scalar1=e1all[:, jb:jb + 1],
                                            scalar2=None, op0=Alu.max)
                    pm = stream.tile([128, NOWN], bf16, tag="pm")
                    nc.vector.tensor_tensor(pm[:], q1[:], mask[:], op=Alu.mult)

                    # evens then odds so consecutive matmuls target
                    # different PSUM banks
                    for ib in [0, 2, 4, 6, 8, 10, 12, 14, 1, 3, 5, 7, 9, 11, 13, 15]:
                        t8, s = divmod(ib, 2)
                        nc.tensor.matmul(
                            acc[t8][:, s * (D + 1):(s + 1) * (D + 1)],
                            lhsT=pm[:, ib * 128:(ib + 1) * 128],
                            rhs=haug[:, jb * (D + 1):(jb + 1) * (D + 1)],
                            start=(jb == 0 and s == 0),
                            stop=(jb == JB - 1 and s == 1),
                            skip_group_check=True)

                # ---- tail: normalize + elu + store ----
                osb = tail_pool.tile([128, NOWN], f32, tag="osb")
                rinv = tail_pool.tile([128, IB], f32, tag="rinv")
                for ib in range(IB):
                    t8, s = divmod(ib, 2)
                    nc.vector.reciprocal(
                        rinv[:, ib:ib + 1],
                        acc[t8][:, s * (D + 1) + D:s * (D + 1) + D + 1])
                for ib in range(IB):
                    t8, s = divmod(ib, 2)
                    nc.vector.tensor_scalar(
                        osb[:, ib * 128:(ib + 1) * 128],
                        in0=acc[t8][:, s * (D + 1):s * (D + 1) + D],
                        scalar1=rinv[:, ib:ib + 1], scalar2=None, op0=Alu.mult)
                # elu(x) = (relu(x) - 1) + exp(min(x, 0))
                zmin = tail_pool.tile([128, NOWN], f32, tag="zmin")
                nc.vector.tensor_scalar(zmin[:], in0=osb[:], scalar1=0.0,
                                        scalar2=None, op0=Alu.min)
                ez = tail_pool.tile([128, NOWN], f32, tag="ez")
                nc.scalar.activation(ez[:], zmin[:], Act.Exp)
                rm1 = tail_pool.tile([128, NOWN], f32, tag="rm1")
                nc.vector.tensor_scalar(rm1[:], in0=osb[:], scalar1=0.0,
                                        scalar2=-1.0, op0=Alu.max, op1=Alu.add)
                oo = tail_pool.tile([128, NOWN], f32, tag="oo")
                nc.vector.tensor_tensor(oo[:], ez[:], rm1[:], op=Alu.add)
                out3 = out_d.rearrange("(b p) d -> p b d", p=128)
                oo3 = oo.rearrange("p (b d) -> p b d", d=D)
                nc.sync.dma_start(out3[:], oo3[:])

    nc.compile()
    return nc


def _host_prep(x, adj, W, a):
    import ml_dtypes
    x = np.asarray(x, np.float32)
    adj = np.asarray(adj)
    W = np.asarray(W, np.float32)
    a = np.asarray(a, np.float32)

    xT = np.ascontiguousarray(x.T.astype(ml_dtypes.bfloat16))          # [F, N]
    adjT = np.ascontiguousarray(adj.T.astype(ml_dtypes.bfloat16))      # [j, i]
    ones1 = np.ones((1, 128), np.float32)

    in_maps = []
    for c in range(M):
        hd, q = divmod(c, 2)
        Wh = W[:, hd * D:(hd + 1) * D]                                  # [F, D]
        wa1 = Wh @ a[:D, 0]
        wa2 = Wh @ a[D:, 0]
        Whc = np.concatenate([Wh, wa1[:, None], wa2[:, None]], axis=1)  # [F, D+2]
        in_maps.append({
            "xT": xT,
            "xq": np.ascontiguousarray(xT[:, q * NOWN:(q + 1) * NOWN]),
            "Wh": np.ascontiguousarray(Whc.astype(ml_dtypes.bfloat16)),
            "maskT": np.ascontiguousarray(adjT[:, q * NOWN:(q + 1) * NOWN]),
            "ones1": ones1,
        })
    return in_maps


def kernel(x, adj, W, a):
    from concourse.bass_utils import run_bass_kernel_spmd

    if "nc" not in _CACHE:
        _CACHE["nc"] = _build_nc()
    nc = _CACHE["nc"]

    in_maps = _host_prep(x, adj, W, a)
    res = run_bass_kernel_spmd(nc, in_maps, list(range(M)))
    out = np.empty((N, H * D), np.float32)
    for c in range(M):
        hd, q = divmod(c, 2)
        out[q * NOWN:(q + 1) * NOWN, hd * D:(hd + 1) * D] = np.asarray(
            res.results[c]["out"], np.float32)
    return out


if __name__ == "__main__":
    nc = _build_nc()
    print("built ok")


# revision 17
# speedup vs baseline: 1.3085x; 1.1138x over previous
"""HGATConv (4-head graph attention, N=4096, F=512) on 8 Trainium2 NeuronCores.

Sharding: (head, node-half) grid — core c handles head c//2 and output rows
q*2048..(q+1)*2048 (q = c%2). Every core computes its head's h = x @ W_h for
ALL nodes locally (bf16 matmuls) — no collective at all.

Attention math per 128-node j-block (tiles are [128 j, 2048 i]): all scores
are divided by exp(s1_i), which is constant per softmax row and cancels in
the normalization:
  p'' = exp(leakyrelu(s1_i + s2_j)) / exp(s1_i)
      = max(exp(-0.8 s1_i + 0.2 s2_j), exp(s2_j))
exp(s2_j) is a per-partition scalar, so the leakyrelu max is a 4x-rate
tensor_scalar; the only wide 2x op left is the adjacency-mask multiply.
Row sums ride along as a ones-column appended to h (129-wide matmuls).

The h-compute loop (stage A) is emitted interleaved with the attention loop
(lag 2) so ACT/DVE/PE pipelines overlap across the two phases; PSUM accs are
packed 3-per-bank (6 banks) so stage A's PSUM pool can coexist.
"""

import sys
import numpy as np

if "/opt/trn_rl_repo" not in sys.path:
    sys.path.insert(0, "/opt/trn_rl_repo")

H, D = 4, 128          # heads, head dim
N, F = 4096, 512       # nodes, features
M = 8                  # cores
NOWN = 1024 * 2        # 2048 own output rows per core
JB = N // 128          # 32 j blocks
IB = NOWN // 128       # 16 own-row blocks
KB = F // 128          # 4 contraction blocks
DA = D + 2             # head W columns + wa2 + wa1
LAG = 2                # h-compute blocks emitted ahead of attention blocks

_CACHE = {}

# attention-matmul emission order: s=0 slices first (their start=True clears
# the bank), consecutive matmuls on different PSUM banks
_MM_ORDER = [0, 3, 6, 9, 12, 15, 1, 4, 7, 10, 13, 2, 5, 8, 11, 14]


def _build_nc():
    import concourse.bacc as bacc
    from concourse import mybir
    from concourse.tile import TileContext

    f32 = mybir.dt.float32
    bf16 = mybir.dt.bfloat16
    Alu = mybir.AluOpType
    Act = mybir.ActivationFunctionType

    nc = bacc.Bacc()
    xT_d = nc.declare_dram_parameter("xT", [F, N], bf16, isOutput=False)
    xq_d = nc.declare_dram_parameter("xq", [F, NOWN], bf16, isOutput=False)
    Wh_d = nc.declare_dram_parameter("Wh", [F, DA], bf16, isOutput=False)
    maskT_d = nc.declare_dram_parameter("maskT", [N, NOWN], bf16, isOutput=False)
    ones1_d = nc.declare_dram_parameter("ones1", [1, 128], f32, isOutput=False)
    out_d = nc.declare_dram_parameter("out", [NOWN, D], f32, isOutput=True)

    with TileContext(nc) as tc:
        with tc.tile_pool(name="const", bufs=1) as const_pool:
            Wh_sb = const_pool.tile([128, KB * DA], bf16)
            for k in range(KB):
                nc.sync.dma_start(Wh_sb[:, k * DA:(k + 1) * DA],
                                  Wh_d[k * 128:(k + 1) * 128, :])
            ones1 = const_pool.tile([1, 128], f32)
            nc.sync.dma_start(ones1[:], ones1_d[:])
            xq = [const_pool.tile([128, NOWN], bf16, name=f"xq{k}") for k in range(KB)]
            for k in range(KB):
                nc.sync.dma_start(xq[k][:], xq_d[k * 128:(k + 1) * 128, :])
            xk = [const_pool.tile([128, N], bf16, name=f"xk{k}") for k in range(KB)]
            for k in range(KB):
                nc.sync.dma_start(xk[k][:], xT_d[k * 128:(k + 1) * 128, :])

            haug = const_pool.tile([128, JB * (D + 1)], bf16)   # [h | 1] per block
            e1all = const_pool.tile([128, JB], f32)             # exp(s2) per block
            sj02 = const_pool.tile([128, JB], f32)              # 0.2*s2 per block
            sT_own = const_pool.tile([1, NOWN], f32)            # s1 row, own cols
            si_b = const_pool.tile([128, NOWN], f32)            # s1 bcast over parts

            # ---- prelude: s1 for own rows, broadcast over partitions ----
            with tc.tile_pool(name="ppsum", bufs=2, space="PSUM") as ppsum:
                for c4 in range(NOWN // 512):
                    pst = ppsum.tile([1, 512], f32, tag="pst")
                    for k in range(KB):
                        nc.tensor.matmul(
                            pst[:],
                            lhsT=Wh_sb[:, k * DA + D + 1:k * DA + D + 2],
                            rhs=xq[k][:, c4 * 512:(c4 + 1) * 512],
                            start=(k == 0), stop=(k == KB - 1))
                    nc.vector.tensor_copy(sT_own[:, c4 * 512:(c4 + 1) * 512], pst[:])
                for t in range(4):
                    pb = ppsum.tile([128, 512], f32, tag=f"pb{t % 2}")
                    for s in range(4):
                        ib = 4 * t + s
                        nc.tensor.matmul(
                            pb[:, s * 128:(s + 1) * 128],
                            lhsT=ones1[:],
                            rhs=sT_own[0:1, ib * 128:(ib + 1) * 128],
                            start=(s == 0), stop=(s == 3),
                            skip_group_check=True)
                    nc.scalar.activation(si_b[:, t * 512:(t + 1) * 512],
                                         pb[:], Act.Copy)

            # ---- merged loop: h-compute block b + attention block b-LAG ----
            with (
                tc.tile_pool(name="acc", bufs=1, space="PSUM") as acc_pool,
                tc.tile_pool(name="hpsum", bufs=2, space="PSUM") as hpsum,
                tc.tile_pool(name="stream", bufs=3) as stream,
                tc.tile_pool(name="tail", bufs=1) as tail_pool,
            ):
                # 5 tiles x 3 slices + 1 tile x 1 slice = 16 accumulators
                acc = [acc_pool.tile([128, 3 * (D + 1)], f32, name=f"acc{t}")
                       for t in range(5)]
                acc.append(acc_pool.tile([128, D + 1], f32, name="acc5"))

                # ones columns for the row-sum ride-along, one strided memset
                haug3 = haug.rearrange("p (b c) -> p b c", c=D + 1)
                nc.vector.memset(haug3[:, :, D:D + 1], 1.0)

                def emit_h_block(b):
                    ph = hpsum.tile([128, D + 1], f32, tag="ph")
                    for k in range(KB):
                        nc.tensor.matmul(
                            ph[:],
                            lhsT=xk[k][:, b * 128:(b + 1) * 128],
                            rhs=Wh_sb[:, k * DA:k * DA + D + 1],
                            start=(k == 0), stop=(k == KB - 1))
                    nc.scalar.activation(haug[:, b * (D + 1):b * (D + 1) + D],
                                         ph[:, 0:D], Act.Copy)
                    nc.scalar.activation(e1all[:, b:b + 1], ph[:, D:D + 1],
                                         Act.Exp)
                    nc.vector.tensor_scalar(sj02[:, b:b + 1], in0=ph[:, D:D + 1],
                                            scalar1=0.2, scalar2=None,
                                            op0=Alu.mult)

                def emit_attn_block(jb):
                    mask = stream.tile([128, NOWN], bf16, tag="mask")
                    nc.sync.dma_start(mask[:], maskT_d[jb * 128:(jb + 1) * 128, :])

                    # t2 = exp(-0.8 s1_i + 0.2 s2_j)
                    t2 = stream.tile([128, NOWN], bf16, tag="t2")
                    nc.scalar.activation(t2[:], si_b[:], Act.Exp,
                                         bias=sj02[:, jb:jb + 1], scale=-0.8)
                    # q1 = max(t2, exp(s2_j))
                    q1 = stream.tile([128, NOWN], bf16, tag="q1")
                    nc.vector.tensor_scalar(q1[:], in0=t2[:],
                                            scalar1=e1all[:, jb:jb + 1],
                                            scalar2=None, op0=Alu.max)
                    pm = stream.tile([128, NOWN], bf16, tag="pm")
                    nc.vector.tensor_tensor(pm[:], q1[:], mask[:], op=Alu.mult)

                    for ib in _MM_ORDER:
                        t8, s = divmod(ib, 3)
                        last = (s == 2) or (t8 == 5)
                        nc.tensor.matmul(
                            acc[t8][:, s * (D + 1):(s + 1) * (D + 1)],
                            lhsT=pm[:, ib * 128:(ib + 1) * 128],
                            rhs=haug[:, jb * (D + 1):(jb + 1) * (D + 1)],
                            start=(jb == 0 and s == 0),
                            stop=(jb == JB - 1 and last),
                            skip_group_check=True)

                for step in range(JB + LAG):
                    if step < JB:
                        emit_h_block(step)
                    if step >= LAG:
                        emit_attn_block(step - LAG)

                # ---- tail: normalize + elu + store ----
                osb = tail_pool.tile([128, NOWN], f32, tag="osb")
                rinv = tail_pool.tile([128, IB], f32, tag="rinv")
                for ib in range(IB):
                    t8, s = divmod(ib, 3)
                    nc.vector.reciprocal(
                        rinv[:, ib:ib + 1],
                        acc[t8][:, s * (D + 1) + D:s * (D + 1) + D + 1])
                for ib in range(IB):
                    t8, s = divmod(ib, 3)
                    nc.vector.tensor_scalar(
                        osb[:, ib * 128:(ib + 1) * 128],
                        in0=acc[t8][:, s * (D + 1):s * (D + 1) + D],
                        scalar1=rinv[:, ib:ib + 1], scalar2=None, op0=Alu.mult)
                # elu(x) = (relu(x) - 1) + exp(min(x, 0))
                zmin = tail_pool.tile([128, NOWN], f32, tag="zmin")
                nc.vector.tensor_scalar(zmin[:], in0=osb[:], scalar1=0.0,
                                        scalar2=None, op0=Alu.min)
                ez = tail_pool.tile([128, NOWN], f32, tag="ez")
                nc.scalar.activation(ez[:], zmin[:], Act.Exp)
                rm1 = tail_pool.tile([128, NOWN], f32, tag="rm1")
                nc.vector.tensor_scalar(rm1[:], in0=osb[:], scalar1=0.0,
                                        scalar2=-1.0, op0=Alu.max, op1=Alu.add)
                oo = tail_pool.tile([128, NOWN], f32, tag="oo")
                nc.vector.tensor_tensor(oo[:], ez[:], rm1[:], op=Alu.add)
                out3 = out_d.rearrange("(b p) d -> p b d", p=128)
                oo3 = oo.rearrange("p (b d) -> p b d", d=D)
                nc.sync.dma_start(out3[:], oo3[:])

    nc.compile()
    return nc


def _host_prep(x, adj, W, a):
    import ml_dtypes
    x = np.asarray(x, np.float32)
    adj = np.asarray(adj)
    W = np.asarray(W, np.float32)
    a = np.asarray(a, np.float32)

    xT = np.ascontiguousarray(x.T.astype(ml_dtypes.bfloat16))          # [F, N]
    adjT = np.ascontiguousarray(adj.T.astype(ml_dtypes.bfloat16))      # [j, i]
    ones1 = np.ones((1, 128), np.float32)

    in_maps = []
    for c in range(M):
        hd, q = divmod(c, 2)
        Wh = W[:, hd * D:(hd + 1) * D]                                  # [F, D]
        wa1 = Wh @ a[:D, 0]
        wa2 = Wh @ a[D:, 0]
        # [W_h | wa2 | wa1]: col D = s2 weights, col D+1 = s1 weights
        Whc = np.concatenate([Wh, wa2[:, None], wa1[:, None]], axis=1)
        in_maps.append({
            "xT": xT,
            "xq": np.ascontiguousarray(xT[:, q * NOWN:(q + 1) * NOWN]),
            "Wh": np.ascontiguousarray(Whc.astype(ml_dtypes.bfloat16)),
            "maskT": np.ascontiguousarray(adjT[:, q * NOWN:(q + 1) * NOWN]),
            "ones1": ones1,
        })
    return in_maps


def kernel(x, adj, W, a):
    from concourse.bass_utils import run_bass_kernel_spmd

    if "nc" not in _CACHE:
        _CACHE["nc"] = _build_nc()
    nc = _CACHE["nc"]

    in_maps = _host_prep(x, adj, W, a)
    res = run_bass_kernel_spmd(nc, in_maps, list(range(M)))
    out = np.empty((N, H * D), np.float32)
    for c in range(M):
        hd, q = divmod(c, 2)
        out[q * NOWN:(q + 1) * NOWN, hd * D:(hd + 1) * D] = np.asarray(
            res.results[c]["out"], np.float32)
    return out


if __name__ == "__main__":
    nc = _build_nc()
    print("built ok")
